# revision 16
# baseline (speedup 1.0000x reference)
"""Trainium2 Bass kernel for DenseCaptioningLoss (nn_DenseCaptioningLoss_38749194944940).

Strategy
--------
The loss depends only on logits rows of ACTIVE tokens (t < len and the
caption item active). The host computes the active-row index list, the
per-row weight 1/len**beta and the target logit (a single element gather
per row), shards the active rows evenly across the 8 cores, and packs each
core's rows contiguously. The device does the heavy part — one pass over
every active [*, V] row with exp(x - K) + fused per-partition accumulate
on the scalar engine.

Device layout: each core's [R, 10000] caption block is viewed as
[R*8, 1250] (identical memory) so every row occupies 8 partition slots.
Chunks of 128 slots stream in as 640KB DMAs, each followed by one
exp+accum that deposits its per-slot sums in a column of a shared
[128, ncc] tile. One PE matmul with a host-provided select matrix
reduces the 8 slots of each row, Ln + two vector ops apply
w * (logZ - tgt), and a second tiny matmul collapses to a scalar.
Program rows (V=2000, <=128 of them) keep a plain row-per-partition
single chunk. IoU runs on 16 intervals per core. Each core emits one
[1, 3] row (cap_sum, prog_sum, iou_sum); the host adds 8 of them.
"""

import ml_dtypes
import numpy as np

import concourse.bass as bass
import concourse.tile as tile
from concourse import mybir
from concourse.bass_utils import run_bass_kernel_spmd
from concourse.vector_clock import ScopedClock

B, C, Lc, Vc = 16, 8, 30, 10000
Lp, Vp = 64, 2000
N_IV = 128
BETA_C = 0.7
BETA_P = 0.7
N_CORES = 8
P = 128
NSLOT = 8  # partition slots per caption row
SEG = Vc // NSLOT  # 1250
F32 = mybir.dt.float32
BF16 = mybir.dt.bfloat16

LAST_RESULTS = None  # BassKernelResults of the most recent run (for test.py)

_patched = [False]


def _patch_tile_drain():
    """This container's walrus build rejects >1 sync-wait on a Drain
    instruction ("Too many sync wait commands"). Split the TileContext
    tail-drain's global-clock waits across multiple single-wait drains."""
    if _patched[0]:
        return
    _patched[0] = True

    def _drain_and_barrier(self, tick_clock, wait_clock):
        nc = self.nc
        drain_inst = nc.sync.drain()
        wait_clock.add_sem_waits(
            drain_inst.ins, ScopedClock({None: tick_clock.global_clock})
        )
        si = drain_inst.ins.sync_info
        if si is not None and si.on_wait and len(si.on_wait) > 1:
            waits = list(si.on_wait)
            si.on_wait = [waits[0]]
            for w in waits[1:]:
                extra = nc.sync.drain()
                esi = extra.ins.sync_info
                if esi is None:
                    extra.ins.sync_info = mybir.SyncInfo(on_wait=[w], on_update=[])
                else:
                    esi.on_wait = [w]
        nc.all_engine_barrier()
        assert self.sems is not None
        popped = nc._tile_sem_poison_stack.pop()
        assert popped is self._sem_poison
        nc.clear_and_free_semaphores(list(self.sems.allocated().values()))

    tile.TileContext._drain_and_barrier = _drain_and_barrier


def _split_multi_waits(nc):
    """This walrus build allows a single sync-wait per instruction; hoist
    extra waits onto same-engine NoOps inserted just before."""
    n_split = 0
    for f in nc.m.functions:
        for bb in f.blocks:
            new_list = []
            changed = False
            for ins in bb.instructions:
                si = ins.sync_info
                if si is not None and si.on_wait and len(si.on_wait) > 1:
                    waits = list(si.on_wait)
                    si.on_wait = [waits[-1]]
                    for w in waits[:-1]:
                        n_split += 1
                        new_list.append(
                            mybir.InstNoOp(
                                name=f"{ins.name}-wsplit-{n_split}",
                                engine=ins.engine,
                                sync_info=mybir.SyncInfo(on_wait=[w], on_update=[]),
                                bass_nofuse=True,
                            )
                        )
                    changed = True
                new_list.append(ins)
            if changed:
                bb.instructions = new_list


def _build(R, Rp, niou, k_cap, k_prog):
    """Per-core SPMD program. R caption rows (as R*8 slots of width 1250),
    Rp program rows, niou interval pairs. k_*: exp stability shifts."""
    Rs = R * NSLOT
    ncc = (Rs + P - 1) // P  # caption slot chunks
    nc = bass.Bass()
    cap_rows = nc.dram_tensor("cap_rows", [Rs, SEG], BF16, kind="ExternalInput")
    cap_meta = nc.dram_tensor("cap_meta", [16, 2 * ncc], F32, kind="ExternalInput")
    prog_rows = nc.dram_tensor("prog_rows", [Rp, Vp], BF16, kind="ExternalInput")
    prog_meta = nc.dram_tensor("prog_meta", [Rp, 2], F32, kind="ExternalInput")
    sel_in = nc.dram_tensor("sel", [P, 17], F32, kind="ExternalInput")
    iou_in = nc.dram_tensor("iou_in", [1, 4 * niou], F32, kind="ExternalInput")
    out = nc.dram_tensor("out", [1, 3], F32, kind="ExternalOutput")

    Exp = mybir.ActivationFunctionType.Exp
    Ln = mybir.ActivationFunctionType.Ln
    Alu = mybir.AluOpType

    # number of caption DMA groups (ramped 1,1,2 then 4s, plus tail)
    n_groups = 0
    s0 = 0
    gi = 0
    while s0 < Rs:
        want = [1, 2][gi] if gi < 2 else 4
        gi += 1
        nj = min(want, (Rs - s0) // P)
        s0 += nj * P if nj >= 1 else Rs - s0
        n_groups += 1

    with tile.TileContext(nc) as tc:
        with (
            tc.tile_pool(name="cappool", bufs=min(max(2, n_groups), 8)) as cappool,
            tc.tile_pool(name="single", bufs=1) as single,
            tc.tile_pool(name="small", bufs=4) as small,
            tc.tile_pool(name="psum", bufs=1, space="PSUM") as psum,
        ):
            # constants computed on otherwise-idle engines
            kb_cap = single.tile([P, 1], F32, tag="kb_cap")
            nc.vector.memset(kb_cap, -k_cap)
            kb_prog = single.tile([P, 1], F32, tag="kb_prog")
            nc.vector.memset(kb_prog, -k_prog)
            # slot sums; 1.0 default keeps Ln finite for never-written slots
            S = single.tile([P, ncc], F32, tag="S")
            nc.vector.memset(S, 1.0)

            # ---- caption slots first on the Sync queue; a small lead
            # group so the scalar engine starts early ----
            ci = 0
            s0 = 0
            gi = 0
            ramp = [1, 2]  # small lead groups fill the pipe; then 4s
            while s0 < Rs:
                want = ramp[gi] if gi < len(ramp) else 4
                gi += 1
                nj = min(want, (Rs - s0) // P)
                if nj >= 1:
                    src_ap = cap_rows[s0 : s0 + nj * P, :].rearrange(
                        "(j p) s -> p j s", p=P
                    )
                    t = cappool.tile([P, nj, SEG], BF16, tag="caprow")
                    # lead groups ride the scalar engine's HWDGE queue so
                    # the first transfers run in parallel with sync's
                    (nc.scalar if gi <= 2 else nc.sync).dma_start(
                        out=t, in_=src_ap
                    )
                    for j in range(nj):
                        nc.scalar.activation(
                            out=t[:, j, :],
                            in_=t[:, j, :],
                            func=Exp,
                            bias=kb_cap,
                            accum_out=S[:, ci + j : ci + j + 1],
                        )
                    ci += nj
                    s0 += nj * P
                else:
                    pc = Rs - s0
                    t2 = cappool.tile([pc, SEG], BF16, tag="captail")
                    nc.sync.dma_start(out=t2, in_=cap_rows[s0 : s0 + pc, :])
                    nc.scalar.activation(
                        out=t2,
                        in_=t2,
                        func=Exp,
                        bias=kb_cap[:pc],
                        accum_out=S[:pc, ci : ci + 1],
                    )
                    ci += 1
                    s0 += pc
            assert ci == ncc

            # ---- program rows + metadata; triggers off the Sync queue ----
            pt = single.tile([Rp, Vp], BF16, tag="progrow")
            nc.sync.dma_start(out=pt, in_=prog_rows[:, :])
            sel_t = single.tile([P, 17], F32, tag="sel")
            nc.gpsimd.dma_start(out=sel_t, in_=sel_in[:, :])
            cm_t = single.tile([16, 2 * ncc], F32, tag="cm")
            nc.gpsimd.dma_start(out=cm_t, in_=cap_meta[:, :])
            pm_t = single.tile([Rp, 2], F32, tag="pm")
            nc.gpsimd.dma_start(out=pm_t, in_=prog_meta[:, :])
            iou_t = single.tile([1, 4 * niou], F32, tag="iou")
            nc.gpsimd.dma_start(out=iou_t, in_=iou_in[:, :])
            ps = small.tile([Rp, 1], F32, tag="ps")
            nc.scalar.activation(
                out=pt, in_=pt, func=Exp, bias=kb_prog[:Rp], accum_out=ps
            )
            plz = small.tile([Rp, 1], F32, tag="plz")
            nc.scalar.activation(out=plz, in_=ps, func=Ln)
            pcon = small.tile([Rp, 1], F32, tag="pcon")
            nc.vector.scalar_tensor_tensor(
                out=pcon,
                in0=plz,
                scalar=pm_t[:, 1:2],
                in1=pm_t[:, 0:1],
                op0=Alu.subtract,
                op1=Alu.mult,
            )
            ppsum = psum.tile([1, 1], F32, tag="ppsum")
            nc.tensor.matmul(ppsum, lhsT=sel_t[:Rp, 16:17], rhs=pcon)
            cpsum = psum.tile([16, ncc], F32, tag="cpsum")
            nc.tensor.matmul(cpsum, lhsT=sel_t[:, 0:16], rhs=S)
            clz = small.tile([16, ncc], F32, tag="clz")
            nc.scalar.activation(out=clz, in_=cpsum, func=Ln)
            cd = small.tile([16, ncc], F32, tag="cd")
            nc.vector.tensor_sub(cd, clz, cm_t[:, ncc : 2 * ncc])
            cm = small.tile([16, ncc], F32, tag="cmm")
            nc.vector.tensor_mul(cm, cd, cm_t[:, 0:ncc])
            cr = small.tile([16, 1], F32, tag="cr")
            nc.vector.reduce_sum(out=cr, in_=cm, axis=mybir.AxisListType.X)
            cpsum2 = psum.tile([1, 1], F32, tag="cpsum2")
            nc.tensor.matmul(cpsum2, lhsT=sel_t[:16, 16:17], rhs=cr)

            # ---- IoU partial: sum(inter/union) over niou intervals ----
            p0 = iou_t[:, 0:niou]
            p1 = iou_t[:, niou : 2 * niou]
            g0 = iou_t[:, 2 * niou : 3 * niou]
            g1 = iou_t[:, 3 * niou : 4 * niou]
            a = small.tile([1, niou], F32, tag="iou_a")
            nc.vector.tensor_tensor(out=a, in0=p1, in1=g1, op=Alu.min)
            b = small.tile([1, niou], F32, tag="iou_b")
            nc.vector.tensor_tensor(out=b, in0=p0, in1=g0, op=Alu.max)
            inter = small.tile([1, niou], F32, tag="iou_i")
            nc.vector.tensor_sub(inter, a, b)
            inter2 = small.tile([1, niou], F32, tag="iou_i2")
            nc.vector.tensor_scalar_max(inter2, inter, 0.0)
            d = small.tile([1, niou], F32, tag="iou_d")
            nc.vector.tensor_tensor(out=d, in0=p1, in1=g1, op=Alu.max)
            e = small.tile([1, niou], F32, tag="iou_e")
            nc.vector.tensor_tensor(out=e, in0=p0, in1=g0, op=Alu.min)
            u = small.tile([1, niou], F32, tag="iou_u")
            nc.vector.tensor_sub(u, d, e)
            ru = small.tile([1, niou], F32, tag="iou_ru")
            nc.vector.reciprocal(ru, u)
            r = small.tile([1, niou], F32, tag="iou_r")
            nc.vector.tensor_mul(r, inter2, ru)
            ssum = small.tile([1, 1], F32, tag="iou_s")
            nc.vector.reduce_sum(out=ssum, in_=r, axis=mybir.AxisListType.X)

            # ---- single [1,3] output row ----
            orow = small.tile([1, 3], F32, tag="orow")
            nc.vector.tensor_copy(orow[:, 0:1], cpsum2)
            nc.vector.tensor_copy(orow[:, 1:2], ppsum)
            nc.vector.tensor_copy(orow[:, 2:3], ssum)
            nc.sync.dma_start(out=out[:, :], in_=orow)
    _split_multi_waits(nc)
    return nc


def _active_rows(logits_flat, tgt_flat, tok_mask_flat, w_flat):
    """Gather active rows + per-row (weight, target logit) metadata,
    split evenly over cores."""
    idx = np.nonzero(tok_mask_flat)[0]
    T = idx.shape[0]
    R = (T + N_CORES - 1) // N_CORES  # rows per core
    pad = R * N_CORES - T
    idx_p = np.concatenate([idx, np.zeros(pad, dtype=idx.dtype)])
    w_p = np.concatenate([w_flat[idx], np.zeros(pad)])
    tgt_p = np.concatenate([tgt_flat[idx], np.zeros(pad, dtype=tgt_flat.dtype)])
    tgt_logit_p = logits_flat[idx_p, tgt_p]
    rows_k, w_k, tl_k = [], [], []
    K = 0.0
    for k in range(N_CORES):
        sl = slice(k * R, (k + 1) * R)
        rows = np.ascontiguousarray(logits_flat[idx_p[sl]], dtype=np.float32)
        K = max(K, float(rows.max(initial=0.0)))
        rows_k.append(rows)
        w_k.append(w_p[sl])
        tl_k.append(tgt_logit_p[sl])
    return rows_k, w_k, tl_k, R, K


def _fold16(vec, ncc):
    """[R] -> [16, ncc] with global row g at [g % 16, g // 16]."""
    out = np.zeros(16 * ncc, dtype=np.float64)
    out[: vec.shape[0]] = vec
    return np.ascontiguousarray(out.reshape(ncc, 16).T)


def kernel(
    gt_captions,
    gt_cap_lens,
    pred_captions,
    gt_program,
    gt_prog_len,
    pred_program,
    gt_intervals,
    pred_intervals,
    gt_caps_count,
    scores,
):
    global LAST_RESULTS
    _patch_tile_drain()

    pred_captions = np.asarray(pred_captions, dtype=np.float32)
    pred_program = np.asarray(pred_program, dtype=np.float32)
    gt_captions = np.asarray(gt_captions).astype(np.int64)
    gt_program = np.asarray(gt_program).astype(np.int64)
    lens_c = np.asarray(gt_cap_lens).astype(np.int64)
    lens_p = np.asarray(gt_prog_len).astype(np.int64)
    counts = np.asarray(gt_caps_count).astype(np.int64)
    gt_iv = np.asarray(gt_intervals, dtype=np.float64).reshape(N_IV, 2)
    pred_iv = np.asarray(pred_intervals, dtype=np.float64).reshape(N_IV, 2)
    scores_np = np.asarray(scores, dtype=np.float64)

    # ----- captions: active rows, weights, target logits -----
    item_mask = np.arange(C)[None, :] < counts[:, None]  # [B, C]
    tok_mask_c = (
        np.arange(Lc)[None, None, :] < lens_c[:, :, None]
    ) & item_mask[:, :, None]
    w_item = np.where(
        item_mask, 1.0 / np.maximum(lens_c, 1).astype(np.float64) ** BETA_C, 0.0
    )
    w_full_c = np.broadcast_to(w_item[:, :, None], (B, C, Lc)).reshape(-1)
    cap_rows_k, cap_w_k, cap_tl_k, R, K_cap = _active_rows(
        pred_captions.reshape(B * C * Lc, Vc),
        gt_captions.reshape(-1),
        tok_mask_c.reshape(-1),
        w_full_c,
    )
    n_items_cap = float(item_mask.sum())

    # ----- program -----
    tok_mask_p = np.arange(Lp)[None, :] < lens_p[:, None]  # [B, Lp]
    w_item_p = 1.0 / np.maximum(lens_p, 1).astype(np.float64) ** BETA_P
    w_full_p = np.broadcast_to(w_item_p[:, None], (B, Lp)).reshape(-1)
    prog_rows_k, prog_w_k, prog_tl_k, Rp, K_prog = _active_rows(
        pred_program.reshape(B * Lp, Vp),
        gt_program.reshape(-1),
        tok_mask_p.reshape(-1),
        w_full_p,
    )

    Rs = R * NSLOT
    ncc = (Rs + P - 1) // P

    # select matrix: col r = 1 where partition//8 == r; col 16 = all ones
    sel = np.zeros((P, 17), dtype=np.float32)
    sel[np.arange(P), np.arange(P) // NSLOT] = 1.0
    sel[:, 16] = 1.0

    niou = N_IV // N_CORES
    in_maps = []
    for k in range(N_CORES):
        # exp stability shift K folded into the target column
        cap_meta = np.concatenate(
            [_fold16(cap_w_k[k], ncc), _fold16(cap_tl_k[k] - K_cap, ncc)], axis=1
        ).astype(np.float32)
        prog_meta = np.zeros((Rp, 2), dtype=np.float32)
        prog_meta[:, 0] = prog_w_k[k]
        prog_meta[:, 1] = prog_tl_k[k] - K_prog
        sl = slice(k * niou, (k + 1) * niou)
        iou_pack = (
            np.concatenate(
                [pred_iv[sl, 0], pred_iv[sl, 1], gt_iv[sl, 0], gt_iv[sl, 1]]
            )
            .astype(np.float32)
            .reshape(1, 4 * niou)
        )
        in_maps.append(
            {
                "cap_rows": cap_rows_k[k].reshape(Rs, SEG).astype(ml_dtypes.bfloat16),
                "cap_meta": cap_meta,
                "prog_rows": prog_rows_k[k].astype(ml_dtypes.bfloat16),
                "prog_meta": prog_meta,
                "sel": sel,
                "iou_in": iou_pack,
            }
        )

    nc = _build(R, Rp, niou, float(K_cap), float(K_prog))
    res = run_bass_kernel_spmd(nc, in_maps, core_ids=list(range(N_CORES)))
    LAST_RESULTS = res

    cap_sum = 0.0
    prog_sum = 0.0
    iou_sum = 0.0
    for k in range(N_CORES):
        o = res.results[k]["out"].astype(np.float64)
        cap_sum += o[0, 0]
        prog_sum += o[0, 1]
        iou_sum += o[0, 2]

    cap_loss = cap_sum / n_items_cap
    prog_loss = prog_sum / float(B)
    iou_loss = 1.0 - iou_sum / float(N_IV)
    loss = (
        scores_np[0] * cap_loss + scores_np[1] * prog_loss + scores_np[2] * iou_loss
    )
    return (
        np.array(loss, dtype=np.float32),
        np.array(cap_loss, dtype=np.float32),
        np.array(prog_loss, dtype=np.float32),
        np.array(iou_loss, dtype=np.float32),
    )
